# revision 15
# baseline (speedup 1.0000x reference)
"""Trainium2 Bass kernel for a dense pre-norm transformer block with ALiBi attention.

Reference semantics (B=2, T=2048, C=1024, H=16, HS=64):
    h  = LN1(x);  q,k,v = per-head projections of h
    wei = softmax(causal(q k^T / sqrt(HS) + alibi))
    x  = x + (concat_heads(wei @ v) @ Wproj + bproj)
    x  = x + (relu(LN2(x) @ W1 + b1) @ W2 + b2)

Distribution over 8 NeuronCores: 2-way data parallel over batch (quads
{0..3} and {4..7}) x 4-way tensor parallel over heads within each quad
(4 heads per core).  The attention out-projection partial sums are
combined with a bf16 token-dim ReduceScatter inside each quad; the FFN
then runs token-parallel (each core owns 512 tokens of its batch).

On-device layout is feature-major ([feature, token]) throughout.  The
host pre-transposes inputs / post-transposes outputs and pre-folds the
LN gains/biases into the adjacent weight matrices.  ALiBi+causal
masking is a multiplicative factor table F[s,t] = exp(-slope*|t-s|) *
(s<=t), precomputed on host per head.  The softmax denominator is
folded into the attention*V matmul by augmenting V with a ones column
(M=65), so no separate denominator matmul pass is needed.
"""

import math

import numpy as np
import ml_dtypes

import concourse.bass as bass
import concourse.mybir as mybir
from concourse import bacc
from concourse.tile import TileContext
from concourse.tile_rust import add_dep_helper
from concourse.bass_utils import run_bass_kernel_spmd

B, T, C, H, HS = 2, 2048, 1024, 16, 64
EPS = 1e-5
NCORES = 8
HPC = 4            # heads per core
TOK = 512          # tokens owned per core (FFN/output shard)
FW = 2432          # factor-table width: 384 + 1536 + 512
BF = mybir.dt.bfloat16
F32 = mybir.dt.float32
AF = mybir.ActivationFunctionType
ALU = mybir.AluOpType
NP_BF16 = ml_dtypes.bfloat16


def _alibi_slopes(n_head):
    n = 2 ** int(math.floor(math.log2(n_head)))
    m = np.power(2.0 ** (-8.0 / n), np.arange(1, n + 1))
    if n < n_head:
        m_hat = np.power(2.0 ** (-4.0 / n), np.arange(1, 1 + 2 * (n_head - n), 2))
        m = np.concatenate([m, m_hat])
    return m.astype(np.float64)


def _factor_table(slope):
    """F[i, u]: for tile (s0, t0), F[i, 384+(t0-s0)+j] = alibi*mask at s=s0+i, t=t0+j."""
    i = np.arange(128)[:, None]
    d = np.arange(FW)[None, :] - 384          # d = (t0-s0)+j;  t-s = d-i
    rel = d - i
    f = np.exp(-slope * np.abs(rel))
    f[rel < 0] = 0.0
    return f.astype(NP_BF16)


def build_bass():
    nc = bacc.Bacc("TRN2", debug=False, num_devices=NCORES)

    # ---- I/O ----
    xfm = nc.dram_tensor("xfm", [128, 8, T], F32, kind="ExternalInput")
    xown = nc.dram_tensor("xown", [128, 8, TOK], F32, kind="ExternalInput")
    wq = nc.dram_tensor("wq", [128, 8, 256], BF, kind="ExternalInput")
    wk = nc.dram_tensor("wk", [128, 8, 256], BF, kind="ExternalInput")
    wv = nc.dram_tensor("wv", [128, 8, 256], BF, kind="ExternalInput")
    bq = nc.dram_tensor("bq", [128, 2], F32, kind="ExternalInput")
    bk = nc.dram_tensor("bk", [128, 2], F32, kind="ExternalInput")
    bv = nc.dram_tensor("bv", [1, 256], F32, kind="ExternalInput")
    wp = nc.dram_tensor("wp", [128, 8, 1024], BF, kind="ExternalInput")
    bp = nc.dram_tensor("bp", [128, 8], F32, kind="ExternalInput")
    ft = nc.dram_tensor("ft", [HPC, 128, FW], BF, kind="ExternalInput")
    w1 = nc.dram_tensor("w1", [32, 128, 8, 128], BF, kind="ExternalInput")
    b1 = nc.dram_tensor("b1", [128, 32], F32, kind="ExternalInput")
    w2 = nc.dram_tensor("w2", [8, 128, 32, 128], BF, kind="ExternalInput")
    b2 = nc.dram_tensor("b2", [128, 8], F32, kind="ExternalInput")
    y = nc.dram_tensor("y", [128, 8, TOK], F32, kind="ExternalOutput")

    with TileContext(nc) as tc:
        with (
            tc.tile_pool(name="const", bufs=1) as cp,
            tc.tile_pool(name="dram", bufs=1, space="DRAM") as dp,
        ):
            ones_bf = cp.tile([128, 1], BF)
            nc.vector.memset(ones_bf[:], 1.0)
            eps_t = cp.tile([1, 1], F32)
            nc.vector.memset(eps_t[:], EPS)
            bq_t = cp.tile([128, 2], F32, tag="bq")
            nc.sync.dma_start(bq_t[:], bq[:])
            bk_t = cp.tile([128, 2], F32, tag="bk")
            nc.sync.dma_start(bk_t[:], bk[:])
            bv_row = cp.tile([1, 256], F32, tag="bvr")
            nc.sync.dma_start(bv_row[:], bv[:])
            bv_b = cp.tile([128, 256], F32, tag="bvb")
            nc.gpsimd.partition_broadcast(bv_b[:], bv_row[:])
            bp_t = cp.tile([128, 8], F32, tag="bp")
            nc.sync.dma_start(bp_t[:], bp[:])
            b1_t = cp.tile([128, 32], F32, tag="b1")
            nc.sync.dma_start(b1_t[:], b1[:])
            b2_t = cp.tile([128, 8], F32, tag="b2")
            nc.sync.dma_start(b2_t[:], b2[:])

            ag_st = []
            for ci in range(4):
                a = dp.tile([2, 128, TOK], BF, tag=f"agst{ci}", name=f"agst{ci}")
                ag_st.append(a)
            ag_all = dp.tile([4, 4, 2, 128, TOK], BF)
            ag_ops = []

            # ---------- LayerNorm (feature-major), writes bf16 out ----------
            # Stats via PE (ones-matmul column sums, sum and sum-of-squares
            # packed into one PSUM bank at partitions 0 / 32); rstd via a
            # single scalar Rsqrt; apply as h = x*rstd_b - (mu*rstd)_b on DVE.
            def ln_stats(x_sb, W, lnps, lnsb):
                # per-kc cast + square feeding interleaved stat matmuls
                st = lnps.tile([33, W], F32, tag="ln_st")
                for kc in range(8):
                    xb = lnsb.tile([128, W], BF, tag="ln_xb", bufs=4)
                    nc.scalar.copy(xb[:], x_sb[:, kc, :])
                    xsq = lnsb.tile([128, W], BF, tag="ln_xsq", bufs=4)
                    nc.scalar.square(xsq[:], x_sb[:, kc, :])
                    nc.tensor.matmul(st[0:1, :], ones_bf[:], xb[:],
                                     start=(kc == 0), stop=(kc == 7),
                                     tile_position=(0, 0),
                                     skip_group_check=True)
                    nc.tensor.matmul(st[32:33, :], ones_bf[:], xsq[:],
                                     start=(kc == 0), stop=(kc == 7),
                                     tile_position=(0, 32),
                                     skip_group_check=True)
                mu = lnsb.tile([1, W], F32, tag="ln_mu")
                nc.scalar.mul(mu[:], st[0:1, :], 1.0 / C)
                msq = lnsb.tile([1, W], F32, tag="ln_msq")
                nc.scalar.mul(msq[:], st[32:33, :], 1.0 / C)
                var = lnsb.tile([1, W], F32, tag="ln_var")
                nc.vector.tensor_tensor(var[:], mu[:], mu[:], ALU.mult)
                nc.vector.tensor_sub(var[:], msq[:], var[:])
                std = lnsb.tile([1, W], F32, tag="ln_std")
                nc.scalar.activation(std[:], var[:], AF.Sqrt, bias=eps_t[:])
                stdb = lnsb.tile([128, W], F32, tag="ln_stdb")
                nc.gpsimd.partition_broadcast(stdb[:], std[:])
                rsb = lnsb.tile([128, W], F32, tag="ln_rsb")
                nc.vector.reciprocal_approx_fast(rsb[:], stdb[:])
                mub = lnsb.tile([128, W], F32, tag="ln_mub")
                nc.gpsimd.partition_broadcast(mub[:], mu[:])
                return rsb, mub

            def ln_apply(x_sb, W, rsb, mub, write_out, lnsb):
                for kc in range(8):
                    tmp = lnsb.tile([128, W], F32, tag="ln_tmp", bufs=3)
                    eng = nc.gpsimd if kc % 2 == 1 else nc.vector
                    eng.tensor_sub(tmp[:], x_sb[:, kc, :], mub[:])
                    nc.vector.tensor_tensor(write_out(kc), tmp[:], rsb[:],
                                            ALU.mult)

            with (
                tc.tile_pool(name="ofmpool", bufs=1) as ofp,
                tc.tile_pool(name="qkvpool", bufs=1) as qp,
            ):
                ofm = ofp.tile([128, 2, T], BF, tag="ofm")
                qfm = qp.tile([128, 2, T], BF, tag="qfm")
                kfm = qp.tile([128, 2, T], BF, tag="kfm")
                v_t = qp.tile([128, 16, HPC, 65], BF, tag="v")
                nc.vector.memset(v_t[:], 1.0)   # ones column at [..., 64]

                with (
                    tc.tile_pool(name="hpool", bufs=1) as hp,
                    tc.tile_pool(name="fpool", bufs=1) as fp,
                ):
                    h_t = hp.tile([128, 8, T], BF, tag="h")
                    f_t = []
                    for hh in range(HPC):
                        f = fp.tile([128, FW], BF, tag=f"ft{hh}")
                        nc.sync.dma_start(f[:], ft[hh])
                        f_t.append(f)


                    # ------- LN1 + QKV + attention, chunk-pipelined -------
                    with (
                        tc.tile_pool(name="xin", bufs=3) as xp,
                        tc.tile_pool(name="ln1ps", bufs=1, space="PSUM") as l1ps,
                        tc.tile_pool(name="ln1sb", bufs=2) as l1sb,
                        tc.tile_pool(name="wqkv", bufs=1) as wqp,
                        tc.tile_pool(name="qkps", bufs=1, space="PSUM") as qps,
                        tc.tile_pool(name="scps", bufs=2, space="PSUM") as scp,
                        tc.tile_pool(name="nups", bufs=1, space="PSUM") as nup,
                        tc.tile_pool(name="attp", bufs=3) as atp,
                        tc.tile_pool(name="onrm", bufs=2) as onp,
                    ):
                        wq_t = wqp.tile([128, 8, 256], BF, tag="wq")
                        nc.sync.dma_start(wq_t[:], wq[:])
                        wk_t = wqp.tile([128, 8, 256], BF, tag="wk")
                        nc.sync.dma_start(wk_t[:], wk[:])
                        wv_t = wqp.tile([128, 8, 256], BF, tag="wv")
                        nc.sync.dma_start(wv_t[:], wv[:])

                        lnres = {}
                        for ch in range(4):
                            xc = xp.tile([128, 8, 512], F32, tag="xc",
                                         name=f"xc{ch}")
                            nc.sync.dma_start(
                                xc[:], xfm[:, :, ch * 512:(ch + 1) * 512])
                            rsb, mub = ln_stats(xc, 512, l1ps, l1sb)
                            lnres[ch] = (xc, rsb, mub)
                            if ch == 0:
                                ln_apply(xc, 512, rsb, mub,
                                         lambda kc: h_t[:, kc, 0:512], l1sb)
                        for ch in range(4):
                            if ch + 1 < 4:
                                xc1, rsb1, mub1 = lnres[ch + 1]
                                ln_apply(
                                    xc1, 512, rsb1, mub1,
                                    lambda kc, ch=ch: h_t[:, kc, (ch + 1) * 512:(ch + 2) * 512],
                                    l1sb)
                            t0 = ch * 512
                            tsl = slice(t0, t0 + 512)
                            # --- QKV for this chunk ---
                            for p in range(2):
                                psq = qps.tile([128, 512], F32, tag="qk")
                                for kc in range(8):
                                    nc.tensor.matmul(
                                        psq[:], wq_t[:, kc, p * 128:(p + 1) * 128],
                                        h_t[:, kc, tsl],
                                        start=(kc == 0), stop=(kc == 7))
                                nc.vector.tensor_scalar_add(
                                    qfm[:, p, tsl], psq[:], bq_t[:, p:p + 1])
                                psk = qps.tile([128, 512], F32, tag="qk")
                                for kc in range(8):
                                    nc.tensor.matmul(
                                        psk[:], wk_t[:, kc, p * 128:(p + 1) * 128],
                                        h_t[:, kc, tsl],
                                        start=(kc == 0), stop=(kc == 7))
                                nc.vector.tensor_scalar_add(
                                    kfm[:, p, tsl], psk[:], bk_t[:, p:p + 1])
                            for tch in range(4):
                                tg = ch * 4 + tch
                                psv = qps.tile([128, 256], F32, tag="qk")
                                for kc in range(8):
                                    nc.tensor.matmul(
                                        psv[:], h_t[:, kc, tg * 128:(tg + 1) * 128],
                                        wv_t[:, kc, :],
                                        start=(kc == 0), stop=(kc == 7))
                                nc.vector.tensor_add(
                                    v_t[:, tg, :, 0:64],
                                    psv[:].rearrange("p (h c) -> p h c", c=64),
                                    bv_b[:].rearrange("p (h c) -> p h c", c=64))

                            # --- attention for query chunk ch, head pairs ---
                            ns = 4 * (ch + 1)
                            for p in range(2):
                                nums = [nup.tile([65, 512], F32, tag=f"num{hh}",
                                                 name=f"num{hh}_{ch}_{p}")
                                        for hh in range(2)]
                                for si in range(ns):
                                    s0 = si * 128
                                    dlt = t0 - s0 + 384
                                    sc = scp.tile([128, 2, 512], F32, tag="sc")
                                    for hh in range(2):
                                        pb = 64 * hh
                                        nc.tensor.matmul(
                                            sc[:, hh, :],
                                            kfm[pb:pb + 64, p, s0:s0 + 128],
                                            qfm[pb:pb + 64, p, tsl],
                                            start=True, stop=True)
                                    at = atp.tile([128, 2, 512], BF, tag="at")
                                    nc.scalar.activation(at[:], sc[:], AF.Exp,
                                                         scale=float(HS) ** -0.5)
                                    for hh in range(2):
                                        head = 2 * p + hh
                                        nc.vector.tensor_tensor(
                                            at[:, hh, :], at[:, hh, :],
                                            f_t[head][:, dlt:dlt + 512], ALU.mult)
                                        nc.tensor.matmul(
                                            nums[hh][:], v_t[:, si, head, :],
                                            at[:, hh, :],
                                            start=(si == 0), stop=(si == ns - 1),
                                            skip_group_check=True)
                                for hh in range(2):
                                    num = nums[hh]
                                    den = onp.tile([1, 512], F32, tag="den")
                                    nc.vector.tensor_copy(den[:], num[64:65, :])
                                    db = onp.tile([64, 512], F32, tag="db")
                                    nc.gpsimd.partition_broadcast(db[:], den[:])
                                    rb = onp.tile([64, 512], F32, tag="rb")
                                    nc.vector.reciprocal_approx_fast(rb[:], db[:])
                                    nc.vector.tensor_tensor(
                                        ofm[64 * hh:64 * hh + 64, p, tsl],
                                        num[0:64, :], rb[:], ALU.mult)

                            # stage this chunk's attention output and AllGather
                            for kc in range(2):
                                nc.sync.dma_start(ag_st[ch][kc],
                                                  ofm[:, kc, tsl])
                            ag = nc.gpsimd.collective_compute(
                                "AllGather", ALU.bypass,
                                replica_groups=[[0, 1, 2, 3], [4, 5, 6, 7]],
                                ins=[ag_st[ch].opt()], outs=[ag_all[ch].opt()])
                            ag_ops.append(ag)

            # ---------- own-slice select + local out-proj + residual ----------
            with tc.tile_pool(name="x2pool", bufs=1) as x2p:
                x2own = x2p.tile([128, 8, TOK], F32, tag="x2own")
                ofa = x2p.tile([128, 8, TOK], BF, tag="ofa")
                xo = x2p.tile([128, 8, TOK], F32, tag="xo")
                nc.sync.dma_start(xo[:], xown[:])
                wp_t = x2p.tile([128, 8, 1024], BF, tag="wp")
                nc.sync.dma_start(wp_t[:], wp[:])
                pid = nc.partition_id()
                own_off = (pid % 4) * (4 * 2 * 128 * TOK)
                base = ag_all[:]
                sel_ap = bass.AP(
                    tensor=base.tensor,
                    offset=own_off,
                    ap=[[TOK, 128], [128 * TOK, 8], [1, TOK]],
                    dep_tracking_offset=0,
                )
                sel = nc.sync.dma_start(ofa[:], sel_ap)
                for ag in ag_ops:
                    add_dep_helper(sel.ins, ag.ins, sync=True,
                                   reason="own-slice select reads all AG outputs")
                with tc.tile_pool(name="prps", bufs=2, space="PSUM") as prp:
                    for m in range(8):
                        ps = prp.tile([128, TOK], F32, tag="pr_ps")
                        for kc in range(8):
                            nc.tensor.matmul(
                                ps[:], wp_t[:, kc, m * 128:(m + 1) * 128],
                                ofa[:, kc, :],
                                start=(kc == 0), stop=(kc == 7))
                        nc.vector.scalar_tensor_tensor(
                            x2own[:, m, :], ps[:], bp_t[:, m:m + 1],
                            xo[:, m, :], ALU.add, ALU.add)

                with (
                    tc.tile_pool(name="ffn", bufs=1) as ffp,
                    tc.tile_pool(name="ln2ps", bufs=1, space="PSUM") as l2ps,
                    tc.tile_pool(name="ln2sb", bufs=2) as l2sb,
                ):
                    h2 = ffp.tile([128, 8, TOK], BF, tag="h2")
                    rsb2, mub2 = ln_stats(x2own, TOK, l2ps, l2sb)
                    ln_apply(x2own, TOK, rsb2, mub2,
                             lambda kc: h2[:, kc, :], l2sb)

                    mid = ffp.tile([128, 32, TOK], BF, tag="mid")
                    with (
                        tc.tile_pool(name="w1p", bufs=4) as w1p,
                        tc.tile_pool(name="ffps", bufs=4, space="PSUM") as fps,
                    ):
                        for m in range(32):
                            w1t = w1p.tile([128, 8, 128], BF, tag="w1t")
                            nc.sync.dma_start(w1t[:], w1[m])
                            ps = fps.tile([128, TOK], F32, tag="ff_ps")
                            for kc in range(8):
                                nc.tensor.matmul(
                                    ps[:], w1t[:, kc, :], h2[:, kc, :],
                                    start=(kc == 0), stop=(kc == 7))
                            nc.scalar.activation(mid[:, m, :], ps[:], AF.Relu,
                                                 bias=b1_t[:, m:m + 1])
                    with (
                        tc.tile_pool(name="w2p", bufs=3) as w2p,
                        tc.tile_pool(name="ff2ps", bufs=4, space="PSUM") as fp2,
                        tc.tile_pool(name="yst", bufs=3) as ysp,
                    ):
                        for m in range(8):
                            w2t = w2p.tile([128, 32, 128], BF, tag="w2t")
                            nc.sync.dma_start(w2t[:], w2[m])
                            ps = fp2.tile([128, TOK], F32, tag="ff2_ps")
                            for kc in range(32):
                                nc.tensor.matmul(
                                    ps[:], w2t[:, kc, :], mid[:, kc, :],
                                    start=(kc == 0), stop=(kc == 31))
                            ym = ysp.tile([128, TOK], F32, tag="ym")
                            nc.vector.scalar_tensor_tensor(
                                ym[:], ps[:], b2_t[:, m:m + 1],
                                x2own[:, m, :], ALU.add, ALU.add)
                            nc.sync.dma_start(y[:, m, :], ym[:])

    nc.compile()
    return nc


_NC_CACHE = None


def _get_nc():
    global _NC_CACHE
    if _NC_CACHE is None:
        _NC_CACHE = build_bass()
    return _NC_CACHE


def _fm_tile(a):
    """[C, N] -> [128, C//128, N] (partition-major feature tiling)."""
    Cd, N = a.shape
    return np.ascontiguousarray(a.reshape(Cd // 128, 128, N).transpose(1, 0, 2))


def prepare_inputs(x, Wq, Wk, Wv, Wproj, bproj, ln1_g, ln1_b, ln2_g, ln2_b,
                   W1, b1, W2, b2):
    """Build the 8 per-core input dicts (all numpy, host side)."""
    x = np.asarray(x, np.float32)
    f32 = lambda a: np.asarray(a, np.float32)
    Wq, Wk, Wv = f32(Wq), f32(Wk), f32(Wv)
    Wproj, bproj = f32(Wproj), f32(bproj)
    ln1_g, ln1_b, ln2_g, ln2_b = f32(ln1_g), f32(ln1_b), f32(ln2_g), f32(ln2_b)
    W1, b1, W2, b2 = f32(W1), f32(b1), f32(W2), f32(b2)

    slopes = _alibi_slopes(H)

    # fold LN1 gain/bias into the QKV weights:  h = ln_raw*g + b
    WqF = Wq * ln1_g[None, :, None]      # [H, C, HS]
    WkF = Wk * ln1_g[None, :, None]
    WvF = Wv * ln1_g[None, :, None]
    bqF = np.einsum("c,hcd->hd", ln1_b, WqF)   # [H, HS]
    bkF = np.einsum("c,hcd->hd", ln1_b, WkF)
    bvF = np.einsum("c,hcd->hd", ln1_b, WvF)
    # fold LN2 gain/bias into W1
    W1F = W1 * ln2_g[:, None]
    b1F = b1 + ln2_b @ W1F

    w1h = np.ascontiguousarray(
        W1F.astype(NP_BF16).reshape(8, 128, 32, 128).transpose(2, 1, 0, 3))
    w2h = np.ascontiguousarray(
        W2.astype(NP_BF16).reshape(32, 128, 8, 128).transpose(2, 1, 0, 3))
    b1h = np.ascontiguousarray(b1F.reshape(32, 128).T)
    b2h = np.ascontiguousarray(b2.reshape(8, 128).T)
    bph = np.ascontiguousarray(bproj.reshape(8, 128).T)
    wph = _fm_tile(Wproj.astype(NP_BF16))      # full Wproj, replicated

    in_maps = []
    for c in range(NCORES):
        b = c // 4
        g = c % 4
        heads = range(4 * g, 4 * g + 4)
        xb = x[b].T                                    # [C, T] feature-major
        wq_own = np.concatenate([WqF[h] for h in heads], axis=1)   # [C, 256]
        wk_own = np.concatenate([WkF[h] for h in heads], axis=1)
        wv_own = np.concatenate([WvF[h] for h in heads], axis=1)
        bq_own = np.concatenate([bqF[h] for h in heads])           # [256]
        bk_own = np.concatenate([bkF[h] for h in heads])
        bv_own = np.concatenate([bvF[h] for h in heads])
        fts = np.stack([_factor_table(slopes[h]) for h in heads])  # [4,128,FW]

        in_maps.append({
            "xfm": _fm_tile(xb),
            "xown": _fm_tile(xb[:, g * TOK:(g + 1) * TOK]),
            "wq": _fm_tile(wq_own.astype(NP_BF16)),
            "wk": _fm_tile(wk_own.astype(NP_BF16)),
            "wv": _fm_tile(wv_own.astype(NP_BF16)),
            "bq": np.ascontiguousarray(bq_own.reshape(2, 128).T.astype(np.float32)),
            "bk": np.ascontiguousarray(bk_own.reshape(2, 128).T.astype(np.float32)),
            "bv": bv_own[None, :].astype(np.float32),
            "wp": wph,
            "bp": bph,
            "ft": fts,
            "w1": w1h,
            "b1": b1h,
            "w2": w2h,
            "b2": b2h,
        })
    return in_maps


def assemble_output(results):
    out = np.empty((B, T, C), np.float32)
    for c in range(NCORES):
        b, g = c // 4, c % 4
        yc = results[c]["y"]                        # [128, 8, TOK]
        yc = yc.transpose(1, 0, 2).reshape(C, TOK)  # [C, TOK]
        out[b, g * TOK:(g + 1) * TOK, :] = yc.T
    return out


def kernel(**inputs):
    nc = _get_nc()
    in_maps = prepare_inputs(**inputs)
    res = run_bass_kernel_spmd(nc, in_maps, core_ids=list(range(NCORES)))
    return assemble_output(res.results)


if __name__ == "__main__":
    import reference
    ins = {k: np.asarray(v) for k, v in reference.setup_inputs().items()}
    exp = np.asarray(reference.reference(**ins))
    got = kernel(**ins)
    err = np.linalg.norm(got - exp) / np.linalg.norm(exp)
    print("Relative error:", err)


# revision 17
# speedup vs baseline: 1.0739x; 1.0739x over previous
"""Trainium2 Bass kernel for a dense pre-norm transformer block with ALiBi attention.

Reference semantics (B=2, T=2048, C=1024, H=16, HS=64):
    h  = LN1(x);  q,k,v = per-head projections of h
    wei = softmax(causal(q k^T / sqrt(HS) + alibi))
    x  = x + (concat_heads(wei @ v) @ Wproj + bproj)
    x  = x + (relu(LN2(x) @ W1 + b1) @ W2 + b2)

Distribution over 8 NeuronCores: 2-way data parallel over batch (quads
{0..3} and {4..7}) x 4-way tensor parallel over heads within each quad
(4 heads per core).  The attention out-projection partial sums are
combined with a bf16 token-dim ReduceScatter inside each quad; the FFN
then runs token-parallel (each core owns 512 tokens of its batch).

On-device layout is feature-major ([feature, token]) throughout.  The
host pre-transposes inputs / post-transposes outputs and pre-folds the
LN gains/biases into the adjacent weight matrices.  ALiBi+causal
masking is a multiplicative factor table F[s,t] = exp(-slope*|t-s|) *
(s<=t), precomputed on host per head.  The softmax denominator is
folded into the attention*V matmul by augmenting V with a ones column
(M=65), so no separate denominator matmul pass is needed.
"""

import math

import numpy as np
import ml_dtypes

import concourse.bass as bass
import concourse.mybir as mybir
from concourse import bacc
from concourse.tile import TileContext
from concourse.tile_rust import add_dep_helper
from concourse.bass_utils import run_bass_kernel_spmd

B, T, C, H, HS = 2, 2048, 1024, 16, 64
EPS = 1e-5
NCORES = 8
HPC = 4            # heads per core
TOK = 512          # tokens owned per core (FFN/output shard)
FW = 2432          # factor-table width: 384 + 1536 + 512
BF = mybir.dt.bfloat16
F32 = mybir.dt.float32
AF = mybir.ActivationFunctionType
ALU = mybir.AluOpType
NP_BF16 = ml_dtypes.bfloat16


def _alibi_slopes(n_head):
    n = 2 ** int(math.floor(math.log2(n_head)))
    m = np.power(2.0 ** (-8.0 / n), np.arange(1, n + 1))
    if n < n_head:
        m_hat = np.power(2.0 ** (-4.0 / n), np.arange(1, 1 + 2 * (n_head - n), 2))
        m = np.concatenate([m, m_hat])
    return m.astype(np.float64)


def _factor_table(slope):
    """F[i, u]: for tile (s0, t0), F[i, 384+(t0-s0)+j] = alibi*mask at s=s0+i, t=t0+j."""
    i = np.arange(128)[:, None]
    d = np.arange(FW)[None, :] - 384          # d = (t0-s0)+j;  t-s = d-i
    rel = d - i
    f = np.exp(-slope * np.abs(rel))
    f[rel < 0] = 0.0
    return f.astype(NP_BF16)


def build_bass():
    nc = bacc.Bacc("TRN2", debug=False, num_devices=NCORES)

    # ---- I/O ----
    xfm = nc.dram_tensor("xfm", [4, 128, 8, 512], F32, kind="ExternalInput")
    xown = nc.dram_tensor("xown", [128, 8, TOK], F32, kind="ExternalInput")
    wq = nc.dram_tensor("wq", [128, 8, 256], BF, kind="ExternalInput")
    wk = nc.dram_tensor("wk", [128, 8, 256], BF, kind="ExternalInput")
    wv = nc.dram_tensor("wv", [128, 8, 256], BF, kind="ExternalInput")
    bq = nc.dram_tensor("bq", [128, 2], F32, kind="ExternalInput")
    bk = nc.dram_tensor("bk", [128, 2], F32, kind="ExternalInput")
    bv = nc.dram_tensor("bv", [1, 256], F32, kind="ExternalInput")
    wp = nc.dram_tensor("wp", [128, 8, 1024], BF, kind="ExternalInput")
    bp = nc.dram_tensor("bp", [128, 8], F32, kind="ExternalInput")
    ft = nc.dram_tensor("ft", [HPC, 128, FW], BF, kind="ExternalInput")
    w1 = nc.dram_tensor("w1", [32, 128, 8, 128], BF, kind="ExternalInput")
    b1 = nc.dram_tensor("b1", [128, 32], F32, kind="ExternalInput")
    w2 = nc.dram_tensor("w2", [8, 128, 32, 128], BF, kind="ExternalInput")
    b2 = nc.dram_tensor("b2", [128, 8], F32, kind="ExternalInput")
    y = nc.dram_tensor("y", [128, 8, TOK], F32, kind="ExternalOutput")

    with TileContext(nc) as tc:
        with (
            tc.tile_pool(name="const", bufs=1) as cp,
            tc.tile_pool(name="dram", bufs=1, space="DRAM") as dp,
        ):
            ones_bf = cp.tile([128, 1], BF)
            nc.vector.memset(ones_bf[:], 1.0)
            eps_t = cp.tile([1, 1], F32)
            nc.vector.memset(eps_t[:], EPS)
            bq_t = cp.tile([128, 2], F32, tag="bq")
            nc.sync.dma_start(bq_t[:], bq[:])
            bk_t = cp.tile([128, 2], F32, tag="bk")
            nc.sync.dma_start(bk_t[:], bk[:])
            bv_row = cp.tile([1, 256], F32, tag="bvr")
            nc.sync.dma_start(bv_row[:], bv[:])
            bv_b = cp.tile([128, 256], F32, tag="bvb")
            nc.gpsimd.partition_broadcast(bv_b[:], bv_row[:])
            bp_t = cp.tile([128, 8], F32, tag="bp")
            nc.sync.dma_start(bp_t[:], bp[:])
            b1_t = cp.tile([128, 32], F32, tag="b1")
            nc.sync.dma_start(b1_t[:], b1[:])
            b2_t = cp.tile([128, 8], F32, tag="b2")
            nc.sync.dma_start(b2_t[:], b2[:])

            ag_st = []
            for ci in range(4):
                a = dp.tile([2, 128, TOK], BF, tag=f"agst{ci}", name=f"agst{ci}")
                ag_st.append(a)
            ag_all = dp.tile([4, 4, 2, 128, TOK], BF)
            ag_ops = []

            # ---------- LayerNorm (feature-major), writes bf16 out ----------
            # Stats via PE (ones-matmul column sums, sum and sum-of-squares
            # packed into one PSUM bank at partitions 0 / 32); rstd via a
            # single scalar Rsqrt; apply as h = x*rstd_b - (mu*rstd)_b on DVE.
            def ln_stats(x_sb, W, lnps, lnsb):
                # per-kc cast + square feeding interleaved stat matmuls
                st = lnps.tile([33, W], F32, tag="ln_st")
                for kc in range(8):
                    xb = lnsb.tile([128, W], BF, tag="ln_xb", bufs=4)
                    nc.scalar.copy(xb[:], x_sb[:, kc, :])
                    xsq = lnsb.tile([128, W], BF, tag="ln_xsq", bufs=4)
                    nc.scalar.square(xsq[:], x_sb[:, kc, :])
                    nc.tensor.matmul(st[0:1, :], ones_bf[:], xb[:],
                                     start=(kc == 0), stop=(kc == 7),
                                     tile_position=(0, 0),
                                     skip_group_check=True)
                    nc.tensor.matmul(st[32:33, :], ones_bf[:], xsq[:],
                                     start=(kc == 0), stop=(kc == 7),
                                     tile_position=(0, 32),
                                     skip_group_check=True)
                mu = lnsb.tile([1, W], F32, tag="ln_mu")
                nc.scalar.mul(mu[:], st[0:1, :], 1.0 / C)
                msq = lnsb.tile([1, W], F32, tag="ln_msq")
                nc.scalar.mul(msq[:], st[32:33, :], 1.0 / C)
                var = lnsb.tile([1, W], F32, tag="ln_var")
                nc.vector.tensor_tensor(var[:], mu[:], mu[:], ALU.mult)
                nc.vector.tensor_sub(var[:], msq[:], var[:])
                std = lnsb.tile([1, W], F32, tag="ln_std")
                nc.scalar.activation(std[:], var[:], AF.Sqrt, bias=eps_t[:])
                stdb = lnsb.tile([128, W], F32, tag="ln_stdb")
                nc.gpsimd.partition_broadcast(stdb[:], std[:])
                rsb = lnsb.tile([128, W], F32, tag="ln_rsb")
                nc.vector.reciprocal_approx_fast(rsb[:], stdb[:])
                mub = lnsb.tile([128, W], F32, tag="ln_mub")
                nc.gpsimd.partition_broadcast(mub[:], mu[:])
                return rsb, mub

            def ln_apply(x_sb, W, rsb, mub, write_out, lnsb):
                for kc in range(8):
                    tmp = lnsb.tile([128, W], F32, tag="ln_tmp", bufs=3)
                    eng = nc.gpsimd if kc % 2 == 1 else nc.vector
                    eng.tensor_sub(tmp[:], x_sb[:, kc, :], mub[:])
                    nc.vector.tensor_tensor(write_out(kc), tmp[:], rsb[:],
                                            ALU.mult)

            with (
                tc.tile_pool(name="ofmpool", bufs=1) as ofp,
                tc.tile_pool(name="qkvpool", bufs=1) as qp,
            ):
                ofm = ofp.tile([128, 2, T], BF, tag="ofm")
                qfm = qp.tile([128, 2, T], BF, tag="qfm")
                kfm = qp.tile([128, 2, T], BF, tag="kfm")
                v_t = qp.tile([128, 16, HPC, 65], BF, tag="v")
                nc.vector.memset(v_t[:], 1.0)   # ones column at [..., 64]

                with (
                    tc.tile_pool(name="hpool", bufs=1) as hp,
                    tc.tile_pool(name="fpool", bufs=1) as fp,
                ):
                    h_t = hp.tile([128, 8, T], BF, tag="h")
                    f_t = []
                    for hh in range(HPC):
                        f = fp.tile([128, FW], BF, tag=f"ft{hh}")
                        nc.sync.dma_start(f[:], ft[hh])
                        f_t.append(f)


                    # ------- LN1 + QKV + attention, chunk-pipelined -------
                    with (
                        tc.tile_pool(name="xin", bufs=2) as xp,
                        tc.tile_pool(name="ln1ps", bufs=1, space="PSUM") as l1ps,
                        tc.tile_pool(name="ln1sb", bufs=2) as l1sb,
                        tc.tile_pool(name="wqkv", bufs=1) as wqp,
                        tc.tile_pool(name="qkps", bufs=1, space="PSUM") as qps,
                        tc.tile_pool(name="scps", bufs=2, space="PSUM") as scp,
                        tc.tile_pool(name="nups", bufs=1, space="PSUM") as nup,
                        tc.tile_pool(name="attp", bufs=3) as atp,
                        tc.tile_pool(name="onrm", bufs=2) as onp,
                    ):
                        wq_t = wqp.tile([128, 8, 256], BF, tag="wq")
                        nc.sync.dma_start(wq_t[:], wq[:])
                        wk_t = wqp.tile([128, 8, 256], BF, tag="wk")
                        nc.sync.dma_start(wk_t[:], wk[:])
                        wv_t = wqp.tile([128, 8, 256], BF, tag="wv")
                        nc.sync.dma_start(wv_t[:], wv[:])

                        lnres = {}
                        for ch in range(4):
                            xc = xp.tile([128, 8, 512], F32, tag="xc",
                                         name=f"xc{ch}")
                            nc.sync.dma_start(xc[:], xfm[ch])
                            rsb, mub = ln_stats(xc, 512, l1ps, l1sb)
                            lnres[ch] = (xc, rsb, mub)
                            if ch == 0:
                                ln_apply(xc, 512, rsb, mub,
                                         lambda kc: h_t[:, kc, 0:512], l1sb)
                        for ch in range(4):
                            if ch + 1 < 4:
                                xc1, rsb1, mub1 = lnres[ch + 1]
                                ln_apply(
                                    xc1, 512, rsb1, mub1,
                                    lambda kc, ch=ch: h_t[:, kc, (ch + 1) * 512:(ch + 2) * 512],
                                    l1sb)
                            t0 = ch * 512
                            tsl = slice(t0, t0 + 512)
                            # --- QKV for this chunk ---
                            for p in range(2):
                                psq = qps.tile([128, 512], F32, tag="qk")
                                for kc in range(8):
                                    nc.tensor.matmul(
                                        psq[:], wq_t[:, kc, p * 128:(p + 1) * 128],
                                        h_t[:, kc, tsl],
                                        start=(kc == 0), stop=(kc == 7))
                                nc.vector.tensor_scalar_add(
                                    qfm[:, p, tsl], psq[:], bq_t[:, p:p + 1])
                                psk = qps.tile([128, 512], F32, tag="qk")
                                for kc in range(8):
                                    nc.tensor.matmul(
                                        psk[:], wk_t[:, kc, p * 128:(p + 1) * 128],
                                        h_t[:, kc, tsl],
                                        start=(kc == 0), stop=(kc == 7))
                                nc.vector.tensor_scalar_add(
                                    kfm[:, p, tsl], psk[:], bk_t[:, p:p + 1])
                            for tch in range(4):
                                tg = ch * 4 + tch
                                psv = qps.tile([128, 256], F32, tag="qk")
                                for kc in range(8):
                                    nc.tensor.matmul(
                                        psv[:], h_t[:, kc, tg * 128:(tg + 1) * 128],
                                        wv_t[:, kc, :],
                                        start=(kc == 0), stop=(kc == 7))
                                nc.vector.tensor_add(
                                    v_t[:, tg, :, 0:64],
                                    psv[:].rearrange("p (h c) -> p h c", c=64),
                                    bv_b[:].rearrange("p (h c) -> p h c", c=64))

                            # --- attention for query chunk ch, head pairs ---
                            ns = 4 * (ch + 1)
                            for p in range(2):
                                nums = [nup.tile([65, 512], F32, tag=f"num{hh}",
                                                 name=f"num{hh}_{ch}_{p}")
                                        for hh in range(2)]
                                for si in range(ns):
                                    s0 = si * 128
                                    dlt = t0 - s0 + 384
                                    sc = scp.tile([128, 2, 512], F32, tag="sc")
                                    for hh in range(2):
                                        pb = 64 * hh
                                        nc.tensor.matmul(
                                            sc[:, hh, :],
                                            kfm[pb:pb + 64, p, s0:s0 + 128],
                                            qfm[pb:pb + 64, p, tsl],
                                            start=True, stop=True)
                                    at = atp.tile([128, 2, 512], BF, tag="at")
                                    nc.scalar.activation(at[:], sc[:], AF.Exp,
                                                         scale=float(HS) ** -0.5)
                                    for hh in range(2):
                                        head = 2 * p + hh
                                        nc.vector.tensor_tensor(
                                            at[:, hh, :], at[:, hh, :],
                                            f_t[head][:, dlt:dlt + 512], ALU.mult)
                                        nc.tensor.matmul(
                                            nums[hh][:], v_t[:, si, head, :],
                                            at[:, hh, :],
                                            start=(si == 0), stop=(si == ns - 1),
                                            skip_group_check=True)
                                for hh in range(2):
                                    num = nums[hh]
                                    den = onp.tile([1, 512], F32, tag="den")
                                    nc.vector.tensor_copy(den[:], num[64:65, :])
                                    db = onp.tile([64, 512], F32, tag="db")
                                    nc.gpsimd.partition_broadcast(db[:], den[:])
                                    rb = onp.tile([64, 512], F32, tag="rb")
                                    nc.vector.reciprocal_approx_fast(rb[:], db[:])
                                    nc.vector.tensor_tensor(
                                        ofm[64 * hh:64 * hh + 64, p, tsl],
                                        num[0:64, :], rb[:], ALU.mult)

                            # stage this chunk's attention output and AllGather
                            for kc in range(2):
                                nc.sync.dma_start(ag_st[ch][kc],
                                                  ofm[:, kc, tsl])
                            ag = nc.gpsimd.collective_compute(
                                "AllGather", ALU.bypass,
                                replica_groups=[[0, 1, 2, 3], [4, 5, 6, 7]],
                                ins=[ag_st[ch].opt()], outs=[ag_all[ch].opt()])
                            ag_ops.append(ag)

            # ---------- own-slice select + local out-proj + residual ----------
            with tc.tile_pool(name="x2pool", bufs=1) as x2p:
                x2own = x2p.tile([128, 8, TOK], F32, tag="x2own")
                ofa = x2p.tile([128, 8, TOK], BF, tag="ofa")
                xo = x2p.tile([128, 8, TOK], F32, tag="xo")
                nc.sync.dma_start(xo[:], xown[:])
                wp_t = x2p.tile([128, 8, 1024], BF, tag="wp")
                nc.sync.dma_start(wp_t[:], wp[:])
                pid = nc.partition_id()
                own_off = (pid % 4) * (4 * 2 * 128 * TOK)
                base = ag_all[:]
                sel_ap = bass.AP(
                    tensor=base.tensor,
                    offset=own_off,
                    ap=[[TOK, 128], [128 * TOK, 8], [1, TOK]],
                    dep_tracking_offset=0,
                )
                sel = nc.sync.dma_start(ofa[:], sel_ap)
                for ag in ag_ops:
                    add_dep_helper(sel.ins, ag.ins, sync=True,
                                   reason="own-slice select reads all AG outputs")
                with tc.tile_pool(name="prps", bufs=2, space="PSUM") as prp:
                    for m in range(8):
                        ps = prp.tile([128, TOK], F32, tag="pr_ps")
                        for kc in range(8):
                            nc.tensor.matmul(
                                ps[:], wp_t[:, kc, m * 128:(m + 1) * 128],
                                ofa[:, kc, :],
                                start=(kc == 0), stop=(kc == 7))
                        nc.vector.scalar_tensor_tensor(
                            x2own[:, m, :], ps[:], bp_t[:, m:m + 1],
                            xo[:, m, :], ALU.add, ALU.add)

                with (
                    tc.tile_pool(name="ffn", bufs=1) as ffp,
                    tc.tile_pool(name="ln2ps", bufs=1, space="PSUM") as l2ps,
                    tc.tile_pool(name="ln2sb", bufs=2) as l2sb,
                ):
                    h2 = ffp.tile([128, 8, TOK], BF, tag="h2")
                    rsb2, mub2 = ln_stats(x2own, TOK, l2ps, l2sb)
                    ln_apply(x2own, TOK, rsb2, mub2,
                             lambda kc: h2[:, kc, :], l2sb)

                    mid = ffp.tile([128, 32, TOK], BF, tag="mid")
                    with (
                        tc.tile_pool(name="w1p", bufs=4) as w1p,
                        tc.tile_pool(name="ffps", bufs=4, space="PSUM") as fps,
                    ):
                        for m in range(32):
                            w1t = w1p.tile([128, 8, 128], BF, tag="w1t")
                            nc.sync.dma_start(w1t[:], w1[m])
                            ps = fps.tile([128, TOK], F32, tag="ff_ps")
                            for kc in range(8):
                                nc.tensor.matmul(
                                    ps[:], w1t[:, kc, :], h2[:, kc, :],
                                    start=(kc == 0), stop=(kc == 7))
                            nc.scalar.activation(mid[:, m, :], ps[:], AF.Relu,
                                                 bias=b1_t[:, m:m + 1])
                    with (
                        tc.tile_pool(name="w2p", bufs=3) as w2p,
                        tc.tile_pool(name="ff2ps", bufs=4, space="PSUM") as fp2,
                        tc.tile_pool(name="yst", bufs=3) as ysp,
                    ):
                        for m in range(8):
                            w2t = w2p.tile([128, 32, 128], BF, tag="w2t")
                            nc.sync.dma_start(w2t[:], w2[m])
                            ps = fp2.tile([128, TOK], F32, tag="ff2_ps")
                            for kc in range(32):
                                nc.tensor.matmul(
                                    ps[:], w2t[:, kc, :], mid[:, kc, :],
                                    start=(kc == 0), stop=(kc == 31))
                            ym = ysp.tile([128, TOK], F32, tag="ym")
                            nc.vector.scalar_tensor_tensor(
                                ym[:], ps[:], b2_t[:, m:m + 1],
                                x2own[:, m, :], ALU.add, ALU.add)
                            nc.sync.dma_start(y[:, m, :], ym[:])

    nc.compile()
    return nc


_NC_CACHE = None


def _get_nc():
    global _NC_CACHE
    if _NC_CACHE is None:
        _NC_CACHE = build_bass()
    return _NC_CACHE


def _fm_tile(a):
    """[C, N] -> [128, C//128, N] (partition-major feature tiling)."""
    Cd, N = a.shape
    return np.ascontiguousarray(a.reshape(Cd // 128, 128, N).transpose(1, 0, 2))


def prepare_inputs(x, Wq, Wk, Wv, Wproj, bproj, ln1_g, ln1_b, ln2_g, ln2_b,
                   W1, b1, W2, b2):
    """Build the 8 per-core input dicts (all numpy, host side)."""
    x = np.asarray(x, np.float32)
    f32 = lambda a: np.asarray(a, np.float32)
    Wq, Wk, Wv = f32(Wq), f32(Wk), f32(Wv)
    Wproj, bproj = f32(Wproj), f32(bproj)
    ln1_g, ln1_b, ln2_g, ln2_b = f32(ln1_g), f32(ln1_b), f32(ln2_g), f32(ln2_b)
    W1, b1, W2, b2 = f32(W1), f32(b1), f32(W2), f32(b2)

    slopes = _alibi_slopes(H)

    # fold LN1 gain/bias into the QKV weights:  h = ln_raw*g + b
    WqF = Wq * ln1_g[None, :, None]      # [H, C, HS]
    WkF = Wk * ln1_g[None, :, None]
    WvF = Wv * ln1_g[None, :, None]
    bqF = np.einsum("c,hcd->hd", ln1_b, WqF)   # [H, HS]
    bkF = np.einsum("c,hcd->hd", ln1_b, WkF)
    bvF = np.einsum("c,hcd->hd", ln1_b, WvF)
    # fold LN2 gain/bias into W1
    W1F = W1 * ln2_g[:, None]
    b1F = b1 + ln2_b @ W1F

    w1h = np.ascontiguousarray(
        W1F.astype(NP_BF16).reshape(8, 128, 32, 128).transpose(2, 1, 0, 3))
    w2h = np.ascontiguousarray(
        W2.astype(NP_BF16).reshape(32, 128, 8, 128).transpose(2, 1, 0, 3))
    b1h = np.ascontiguousarray(b1F.reshape(32, 128).T)
    b2h = np.ascontiguousarray(b2.reshape(8, 128).T)
    bph = np.ascontiguousarray(bproj.reshape(8, 128).T)
    wph = _fm_tile(Wproj.astype(NP_BF16))      # full Wproj, replicated

    in_maps = []
    for c in range(NCORES):
        b = c // 4
        g = c % 4
        heads = range(4 * g, 4 * g + 4)
        xb = x[b].T                                    # [C, T] feature-major
        wq_own = np.concatenate([WqF[h] for h in heads], axis=1)   # [C, 256]
        wk_own = np.concatenate([WkF[h] for h in heads], axis=1)
        wv_own = np.concatenate([WvF[h] for h in heads], axis=1)
        bq_own = np.concatenate([bqF[h] for h in heads])           # [256]
        bk_own = np.concatenate([bkF[h] for h in heads])
        bv_own = np.concatenate([bvF[h] for h in heads])
        fts = np.stack([_factor_table(slopes[h]) for h in heads])  # [4,128,FW]

        xfm_t = _fm_tile(xb)                           # [128, 8, 2048]
        xfm_c = np.ascontiguousarray(
            xfm_t.reshape(128, 8, 4, 512).transpose(2, 0, 1, 3))
        in_maps.append({
            "xfm": xfm_c,
            "xown": _fm_tile(xb[:, g * TOK:(g + 1) * TOK]),
            "wq": _fm_tile(wq_own.astype(NP_BF16)),
            "wk": _fm_tile(wk_own.astype(NP_BF16)),
            "wv": _fm_tile(wv_own.astype(NP_BF16)),
            "bq": np.ascontiguousarray(bq_own.reshape(2, 128).T.astype(np.float32)),
            "bk": np.ascontiguousarray(bk_own.reshape(2, 128).T.astype(np.float32)),
            "bv": bv_own[None, :].astype(np.float32),
            "wp": wph,
            "bp": bph,
            "ft": fts,
            "w1": w1h,
            "b1": b1h,
            "w2": w2h,
            "b2": b2h,
        })
    return in_maps


def assemble_output(results):
    out = np.empty((B, T, C), np.float32)
    for c in range(NCORES):
        b, g = c // 4, c % 4
        yc = results[c]["y"]                        # [128, 8, TOK]
        yc = yc.transpose(1, 0, 2).reshape(C, TOK)  # [C, TOK]
        out[b, g * TOK:(g + 1) * TOK, :] = yc.T
    return out


def kernel(**inputs):
    nc = _get_nc()
    in_maps = prepare_inputs(**inputs)
    res = run_bass_kernel_spmd(nc, in_maps, core_ids=list(range(NCORES)))
    return assemble_output(res.results)


if __name__ == "__main__":
    import reference
    ins = {k: np.asarray(v) for k, v in reference.setup_inputs().items()}
    exp = np.asarray(reference.reference(**ins))
    got = kernel(**ins)
    err = np.linalg.norm(got - exp) / np.linalg.norm(exp)
    print("Relative error:", err)
